# revision 5
# baseline (speedup 1.0000x reference)
"""GCGRU cell (graph-conv GRU, diffusion-conv gates) on 8 TRN2 NeuronCores.

Math (per batch b, N=1024 nodes, D=2 in-feats, U=64 units, S=2 supports):
  x0   = [inputs_b | H_b]                          (N, 66)
  for gate g in {r, u, c}:
    pre_g = x0g @ Wg_m0 + sum_s A_s @ (x0g @ Wg_{m=s+1}) + bias_g
  (reassociated: (A_s @ x0) @ W == A_s @ (x0 @ W), so the N x N matrices
   multiply a tiny (N, 64) matrix instead of (N, 66) twice per gate)
  r, u = sigmoid(pre_r), sigmoid(pre_u); c = tanh(pre_c with x0c=[inputs|r*H])
  h = u * H + (1 - u) * c

Implementation notes:
  - Data parallel over batch: 32 batches -> 4 per core, no collectives.
  - supports[b] is loaded in natural layout (i on partitions, (j,s) on free:
    full-bandwidth contiguous HBM reads), cast f32->bf16 on GPSIMD, then
    transposed 128x128 tile-wise on the TensorEngine (transpose-mode matmul
    with a bf16 identity), de-interleaving s via a stride-2 free AP.  The
    transposed A_s^T tiles (j on partitions) are the moving operand of the
    big matmuls; everything accumulates in f32 PSUM.
  - All PSUM results drain through the VectorEngine; sigmoids/tanh run on
    the ScalarEngine straight out of PSUM; gate elementwise math stays f32.
"""

import numpy as np

import concourse.bacc as bacc
import concourse.mybir as mybir
import concourse.tile as tile
from concourse.bass_utils import run_bass_kernel_spmd
from concourse.masks import make_identity

B, N, D, U, S = 32, 1024, 2, 64, 2
F = D + U                      # 66
NCORES = 8
BPC = B // NCORES              # 4 batches per core
P = 128                        # partitions
JB = N // P                    # 8 j-blocks per support
IT = N // P                    # 8 i-tiles per support
F32 = mybir.dt.float32
BF16 = mybir.dt.bfloat16

_COMPILED = {}


def _build():
    nc = bacc.Bacc("TRN2", target_bir_lowering=False, debug=False)

    t_inputs = nc.dram_tensor("inputs", [BPC, N, D], F32, kind="ExternalInput")
    t_supports = nc.dram_tensor("supports", [BPC, N, N, S], F32, kind="ExternalInput")
    t_hprev = nc.dram_tensor("h_prev", [BPC, N * U], F32, kind="ExternalInput")
    t_wk = {g: nc.dram_tensor(f"{g}_kernel", [F * 3, U], F32, kind="ExternalInput")
            for g in "ruc"}
    t_wb = {g: nc.dram_tensor(f"{g}_bias", [U], F32, kind="ExternalInput")
            for g in "ruc"}
    t_out = nc.dram_tensor("out", [BPC, N * U], F32, kind="ExternalOutput")

    with tile.TileContext(nc) as tc:
        with (
            tc.tile_pool(name="const", bufs=1) as constp,
            tc.tile_pool(name="wt", bufs=1) as wtp,
            tc.tile_pool(name="stage", bufs=3) as stagep,
            tc.tile_pool(name="abf", bufs=3) as abfp,
            tc.tile_pool(name="at", bufs=2) as atp,
            tc.tile_pool(name="act", bufs=2) as actp,
            tc.tile_pool(name="psA", bufs=4, space="PSUM") as psA,
            tc.tile_pool(name="psB", bufs=3, space="PSUM") as psB,
        ):
            # ---- constants ----
            id_bf = constp.tile([P, P], BF16, tag="id_bf")
            make_identity(nc, id_bf[:])
            id_f32 = constp.tile([P, P], F32, tag="id_f32")
            make_identity(nc, id_f32[:])

            # ---- gate weights, rearranged into hop blocks, cast to bf16 ----
            # W rows are (f, m) pairs, m fastest: row f*3 + m.
            # x0 feature order is [H (64) | inputs (2)] so every partition
            # slice starts at 0 or 64; W rows are permuted to match.
            def load_w_block(g, m, dst_ap):
                st = wtp.tile([F, U], F32, tag=f"wst_{g}{m}", name=f"wst_{g}{m}")
                src = t_wk[g].ap().rearrange("(f three) u -> f three u", three=3)
                nc.sync.dma_start(st[0:U, :], src[D:F, m, :])
                nc.sync.dma_start(st[U:F, :], src[0:D, m, :])
                nc.gpsimd.tensor_copy(dst_ap, st[:])

            w0ru = wtp.tile([F, 2 * U], BF16, tag="w0ru")
            load_w_block("r", 0, w0ru[:, 0:U])
            load_w_block("u", 0, w0ru[:, U:2 * U])
            wru_s = []
            for s in range(S):
                w = wtp.tile([F, 2 * U], BF16, tag=f"wru{s}")
                load_w_block("r", s + 1, w[:, 0:U])
                load_w_block("u", s + 1, w[:, U:2 * U])
                wru_s.append(w)
            wc0 = wtp.tile([F, U], BF16, tag="wc0")
            load_w_block("c", 0, wc0[:])
            wc_s = []
            for s in range(S):
                w = wtp.tile([F, U], BF16, tag=f"wcs{s}")
                load_w_block("c", s + 1, w[:])
                wc_s.append(w)

            bias = {}
            for g in "ruc":
                bt = wtp.tile([U, 1], F32, tag=f"bias_{g}")
                nc.sync.dma_start(bt[:], t_wb[g].ap().rearrange("(u one) -> u one", one=1))
                bias[g] = bt

            for b in range(BPC):
                # ---- load + cast + transpose supports[b] ----
                # AT[s][q, jb*N + i] = A_s[i, jb*128 + q]   (bf16)
                at = [atp.tile([P, JB * N], BF16, tag=f"at{s}", name=f"at{s}")
                      for s in range(S)]
                for it in range(IT):
                    stg = stagep.tile([P, N * S], F32, tag="stage")
                    nc.sync.dma_start(stg[:], t_supports.ap()[b, it * P:(it + 1) * P])
                    ab = abfp.tile([P, N * S], BF16, tag="abf")
                    nc.gpsimd.tensor_copy(ab[:], stg[:])
                    ab3 = ab[:].rearrange("p (j two) -> p j two", two=2)
                    for s in range(S):
                        for jb in range(JB):
                            pt = psA.tile([P, P], BF16, tag="psA")
                            nc.tensor.transpose(
                                pt[:], ab3[:, jb * P:(jb + 1) * P, s], id_bf[:])
                            nc.vector.tensor_copy(
                                at[s][:, jb * N + it * P: jb * N + (it + 1) * P],
                                pt[:])

                # ---- x0 = [inputs | H], transposed (x0T) in f32 + bf16 ----
                x0n = actp.tile([P, JB * F], F32, tag="x0n")
                nc.sync.dma_start(
                    x0n[:].rearrange("p (jb f) -> p jb f", f=F)[:, :, 0:U],
                    t_hprev.ap()[b].rearrange("(jb p u) -> p jb u", p=P, u=U))
                nc.sync.dma_start(
                    x0n[:].rearrange("p (jb f) -> p jb f", f=F)[:, :, U:F],
                    t_inputs.ap()[b].rearrange("(jb p) d -> p jb d", p=P))
                x0T = actp.tile([F, N], F32, tag="x0T")
                x0Tb = actp.tile([F, N], BF16, tag="x0Tb")
                for jb in range(JB):
                    px = psA.tile([F, P], F32, tag="psA")
                    nc.tensor.transpose(
                        px[:], x0n[:, jb * F:(jb + 1) * F], id_f32[:])
                    nc.vector.tensor_copy(x0T[:, jb * P:(jb + 1) * P], px[:])
                    nc.vector.tensor_copy(x0Tb[:, jb * P:(jb + 1) * P], px[:])

                # ---- Z_ru_s = x0 @ [Wr_{s+1} | Wu_{s+1}]  (N, 128) bf16 ----
                zru = []
                for s in range(S):
                    z = actp.tile([P, JB * 2 * U], BF16, tag=f"zru{s}")
                    for jb in range(JB):
                        pz = psA.tile([P, 2 * U], F32, tag="psA")
                        nc.tensor.matmul(
                            pz[:], x0Tb[:, jb * P:(jb + 1) * P], wru_s[s][:],
                            start=True, stop=True)
                        nc.vector.tensor_copy(
                            z[:, jb * 2 * U:(jb + 1) * 2 * U], pz[:])
                    zru.append(z)

                # ---- phase 1: pre_ru^T = sum_s A_s^T-weighted + x0 term ----
                rT = actp.tile([U, N], BF16, tag="rT")
                uT = actp.tile([U, N], F32, tag="uT")
                NC2 = N // 2
                for ic in range(2):
                    p1 = psB.tile([P, NC2], F32, tag="psB")
                    k = 0
                    for s in range(S):
                        for jb in range(JB):
                            nc.tensor.matmul(
                                p1[:],
                                zru[s][:, jb * 2 * U:(jb + 1) * 2 * U],
                                at[s][:, jb * N + ic * NC2: jb * N + (ic + 1) * NC2],
                                start=(k == 0), stop=False)
                            k += 1
                    nc.tensor.matmul(
                        p1[:], w0ru[:], x0Tb[:, ic * NC2:(ic + 1) * NC2],
                        start=False, stop=True)
                    nc.scalar.activation(
                        rT[:, ic * NC2:(ic + 1) * NC2], p1[0:U, :],
                        mybir.ActivationFunctionType.Sigmoid, bias=bias["r"][:])
                    nc.scalar.activation(
                        uT[:, ic * NC2:(ic + 1) * NC2], p1[U:2 * U, :],
                        mybir.ActivationFunctionType.Sigmoid, bias=bias["u"][:])

                # ---- x0c^T = [inputs^T | (r * H)^T] (bf16) ----
                x0cT = actp.tile([F, N], BF16, tag="x0cT")
                nc.vector.tensor_copy(x0cT[U:F, :], x0Tb[U:F, :])
                nc.vector.tensor_mul(x0cT[0:U, :], rT[:], x0T[0:U, :])

                # ---- Z_c_s = x0c @ Wc_{s+1}  (N, 64) bf16 ----
                zc = []
                for s in range(S):
                    z = actp.tile([P, JB * U], BF16, tag=f"zc{s}")
                    for jb in range(JB):
                        pz = psA.tile([P, U], F32, tag="psA")
                        nc.tensor.matmul(
                            pz[:], x0cT[:, jb * P:(jb + 1) * P], wc_s[s][:],
                            start=True, stop=True)
                        nc.vector.tensor_copy(z[:, jb * U:(jb + 1) * U], pz[:])
                    zc.append(z)

                # ---- phase 2: pre_c^T ----
                cT = actp.tile([U, N], F32, tag="cT")
                for ic in range(2):
                    p2 = psB.tile([U, NC2], F32, tag="psB")
                    k = 0
                    for s in range(S):
                        for jb in range(JB):
                            nc.tensor.matmul(
                                p2[:],
                                zc[s][:, jb * U:(jb + 1) * U],
                                at[s][:, jb * N + ic * NC2: jb * N + (ic + 1) * NC2],
                                start=(k == 0), stop=False)
                            k += 1
                    nc.tensor.matmul(
                        p2[:], wc0[:], x0cT[:, ic * NC2:(ic + 1) * NC2],
                        start=False, stop=True)
                    nc.scalar.activation(
                        cT[:, ic * NC2:(ic + 1) * NC2], p2[:],
                        mybir.ActivationFunctionType.Tanh, bias=bias["c"][:])

                # ---- h^T = c^T + u^T * (H^T - c^T);  back to natural ----
                hT = actp.tile([U, N], F32, tag="hT")
                nc.vector.tensor_sub(hT[:], x0T[0:U, :], cT[:])
                nc.vector.tensor_mul(hT[:], hT[:], uT[:])
                nc.vector.tensor_add(hT[:], hT[:], cT[:])
                hnat = actp.tile([P, JB * U], F32, tag="hnat")
                for jb in range(JB):
                    ph = psA.tile([P, U], F32, tag="psA")
                    nc.tensor.transpose(
                        ph[:], hT[:, jb * P:(jb + 1) * P], id_f32[0:U, 0:U])
                    nc.vector.tensor_copy(hnat[:, jb * U:(jb + 1) * U], ph[:])
                nc.sync.dma_start(
                    t_out.ap()[b].rearrange("(jb p u) -> p jb u", p=P, u=U),
                    hnat[:].rearrange("p (jb u) -> p jb u", u=U))

    nc.finalize()
    return nc


def _make_in_maps(inputs):
    in_maps = []
    for c in range(NCORES):
        lo, hi = c * BPC, (c + 1) * BPC
        in_maps.append({
            "inputs": np.ascontiguousarray(inputs["inputs"][lo:hi], np.float32),
            "supports": np.ascontiguousarray(inputs["supports"][lo:hi], np.float32),
            "h_prev": np.ascontiguousarray(inputs["h_prev"][lo:hi], np.float32),
            "r_kernel": np.ascontiguousarray(inputs["r_kernel"], np.float32),
            "u_kernel": np.ascontiguousarray(inputs["u_kernel"], np.float32),
            "c_kernel": np.ascontiguousarray(inputs["c_kernel"], np.float32),
            "r_bias": np.ascontiguousarray(inputs["r_bias"], np.float32),
            "u_bias": np.ascontiguousarray(inputs["u_bias"], np.float32),
            "c_bias": np.ascontiguousarray(inputs["c_bias"], np.float32),
        })
    return in_maps


def kernel(**inputs):
    nc = _COMPILED.get("nc")
    if nc is None:
        nc = _COMPILED["nc"] = _build()

    res = run_bass_kernel_spmd(nc, _make_in_maps(inputs), core_ids=list(range(NCORES)))
    out = np.concatenate([res.results[c]["out"] for c in range(NCORES)], axis=0)
    return out.astype(np.float32)


# revision 9
# speedup vs baseline: 1.3724x; 1.3724x over previous
"""GCGRU cell (graph-conv GRU, diffusion-conv gates) on 8 TRN2 NeuronCores.

Math (per batch b, N=1024 nodes, D=2 in-feats, U=64 units, S=2 supports):
  x0   = [H_b | inputs_b]                          (N, 66)  (feature-permuted)
  for gate g in {r, u, c}:
    pre_g = x0g @ Wg_m0 + sum_s A_s @ (x0g @ Wg_{m=s+1}) + bias_g
  (reassociated: (A_s @ x0) @ W == A_s @ (x0 @ W), so the N x N supports
   multiply a tiny (N, 64) matrix instead of the other association order)
  r, u = sigmoid(pre_r), sigmoid(pre_u); c = tanh(pre_c with x0c=[r*H|inputs])
  h = u * H + (1 - u) * c

Implementation notes:
  - Data parallel over batch: 32 batches -> 4 per core, no collectives.
  - supports[b] is cast f32->bf16 *during* the HBM->SBUF DMA (SWDGE cast,
    ~0.86x plain-DMA rate) in natural layout (i on partitions, (j,s) free).
  - The j-contraction needs j on partitions, so A is transposed 128x128
    tile-wise on the TensorEngine (transpose-mode matmul with a bf16
    identity), de-interleaving s via a stride-2 free AP.  Four transposed
    tiles share one [128,512] bf16 PSUM tile (one accumulation group) so
    the VectorEngine drains them in one op.
  - Gate pre-activations accumulate in f32 PSUM with the transposed A_s
    tiles as the moving operand; sigmoid/tanh run on the ScalarEngine
    straight out of PSUM; small drains go to ScalarE, gate elementwise to
    GpSimd, keeping the VectorEngine for the big transpose drains.
"""

import numpy as np

import concourse.bacc as bacc
import concourse.mybir as mybir
import concourse.tile as tile
from concourse.bass_utils import run_bass_kernel_spmd
from concourse.masks import make_identity

B, N, D, U, S = 32, 1024, 2, 64, 2
F = D + U                      # 66
NCORES = 8
BPC = B // NCORES              # 4 batches per core
P = 128                        # partitions
JB = N // P                    # 8 j-blocks per support
F32 = mybir.dt.float32
BF16 = mybir.dt.bfloat16

_COMPILED = {}


def _build():
    nc = bacc.Bacc("TRN2", target_bir_lowering=False, debug=False)

    t_inputs = nc.dram_tensor("inputs", [BPC, N, D], F32, kind="ExternalInput")
    t_supports = nc.dram_tensor("supports", [BPC, N, N, S], F32, kind="ExternalInput")
    t_hprev = nc.dram_tensor("h_prev", [BPC, N * U], F32, kind="ExternalInput")
    t_wk = {g: nc.dram_tensor(f"{g}_kernel", [F * 3, U], F32, kind="ExternalInput")
            for g in "ruc"}
    t_wb = {g: nc.dram_tensor(f"{g}_bias", [U], F32, kind="ExternalInput")
            for g in "ruc"}
    t_out = nc.dram_tensor("out", [BPC, N * U], F32, kind="ExternalOutput")

    QC = 4                 # i-tiles per load chunk
    NCH = N // (QC * P)    # 2 chunks per batch

    with tile.TileContext(nc) as tc:
        with (
            tc.tile_pool(name="const", bufs=1) as constp,
            tc.tile_pool(name="wt", bufs=1) as wtp,
            tc.tile_pool(name="abf", bufs=3) as abfp,
            tc.tile_pool(name="at", bufs=2) as atp,
            tc.tile_pool(name="act", bufs=2) as actp,
            tc.tile_pool(name="psA", bufs=4, space="PSUM") as psA,
            tc.tile_pool(name="psB", bufs=3, space="PSUM") as psB,
        ):
            # ---- constants ----
            id_bf = constp.tile([P, P], BF16, tag="id_bf")
            make_identity(nc, id_bf[:])
            id_f32 = constp.tile([P, P], F32, tag="id_f32")
            make_identity(nc, id_f32[:])

            # ---- gate weights, hop blocks, permuted to [H|inputs], bf16 ----
            # W rows are (f, m) pairs, m fastest: row f*3 + m.
            def load_w_block(g, m, dst_ap):
                st = wtp.tile([F, U], F32, tag=f"wst_{g}{m}", name=f"wst_{g}{m}")
                src = t_wk[g].ap().rearrange("(f three) u -> f three u", three=3)
                nc.sync.dma_start(st[0:U, :], src[D:F, m, :])
                nc.sync.dma_start(st[U:F, :], src[0:D, m, :])
                nc.gpsimd.tensor_copy(dst_ap, st[:])

            w0ru = wtp.tile([F, 2 * U], BF16, tag="w0ru")
            load_w_block("r", 0, w0ru[:, 0:U])
            load_w_block("u", 0, w0ru[:, U:2 * U])
            wru_s = []
            for s in range(S):
                w = wtp.tile([F, 2 * U], BF16, tag=f"wru{s}")
                load_w_block("r", s + 1, w[:, 0:U])
                load_w_block("u", s + 1, w[:, U:2 * U])
                wru_s.append(w)
            wc0 = wtp.tile([F, U], BF16, tag="wc0")
            load_w_block("c", 0, wc0[:])
            wc_s = []
            for s in range(S):
                w = wtp.tile([F, U], BF16, tag=f"wcs{s}")
                load_w_block("c", s + 1, w[:])
                wc_s.append(w)

            bias = {}
            for g in "ruc":
                bt = wtp.tile([U, 1], F32, tag=f"bias_{g}")
                nc.sync.dma_start(bt[:], t_wb[g].ap().rearrange("(u one) -> u one", one=1))
                bias[g] = bt

            sup4 = t_supports.ap().rearrange(
                "b (q p) j two -> b p q (j two)", p=P)

            for b in range(BPC):
                # ---- load (cast f32->bf16 in-DMA) + transpose supports[b] ----
                # AT[s][q, jb*N + i] = A_s[i, jb*128 + q]   (bf16)
                at = [atp.tile([P, JB * N], BF16, tag=f"at{s}", name=f"at{s}")
                      for s in range(S)]
                for ch in range(NCH):
                    ab = abfp.tile([P, QC * N * S], BF16, tag="abf")
                    nc.gpsimd.dma_start(
                        ab[:], sup4[b, :, ch * QC:(ch + 1) * QC, :])
                    ab4 = ab[:].rearrange("p (q j two) -> p q j two", q=QC, two=2)
                    for s in range(S):
                        for jb in range(JB):
                            pt4 = psA.tile([P, QC * P], BF16, tag="psA", bufs=3)
                            for q in range(QC):
                                nc.tensor.matmul(
                                    pt4[:, q * P:(q + 1) * P],
                                    ab4[:, q, jb * P:(jb + 1) * P, s],
                                    id_bf[:],
                                    start=(q == 0), stop=(q == QC - 1),
                                    is_transpose=True)
                            nc.vector.tensor_copy(
                                at[s][:, jb * N + ch * QC * P:
                                      jb * N + (ch + 1) * QC * P],
                                pt4[:])

                # ---- x0 = [H | inputs], transposed (x0T) in f32 + bf16 ----
                x0n = actp.tile([P, JB * F], F32, tag="x0n")
                nc.sync.dma_start(
                    x0n[:].rearrange("p (jb f) -> p jb f", f=F)[:, :, 0:U],
                    t_hprev.ap()[b].rearrange("(jb p u) -> p jb u", p=P, u=U))
                nc.sync.dma_start(
                    x0n[:].rearrange("p (jb f) -> p jb f", f=F)[:, :, U:F],
                    t_inputs.ap()[b].rearrange("(jb p) d -> p jb d", p=P))
                x0T = actp.tile([F, N], F32, tag="x0T")
                x0Tb = actp.tile([F, N], BF16, tag="x0Tb")
                for jb in range(JB):
                    px = psA.tile([F, P], F32, tag="psAx", bufs=2)
                    nc.tensor.transpose(
                        px[:], x0n[:, jb * F:(jb + 1) * F], id_f32[:])
                    nc.scalar.copy(x0T[:, jb * P:(jb + 1) * P], px[:])
                    nc.scalar.copy(x0Tb[:, jb * P:(jb + 1) * P], px[:])

                # ---- Z_ru_s = x0 @ [Wr_{s+1} | Wu_{s+1}]  (N, 128) bf16 ----
                zru = []
                for s in range(S):
                    z = actp.tile([P, JB * 2 * U], BF16, tag=f"zru{s}")
                    for jb in range(JB):
                        pz = psA.tile([P, 2 * U], F32, tag="psAx", bufs=2)
                        nc.tensor.matmul(
                            pz[:], x0Tb[:, jb * P:(jb + 1) * P], wru_s[s][:],
                            start=True, stop=True)
                        nc.scalar.copy(z[:, jb * 2 * U:(jb + 1) * 2 * U], pz[:])
                    zru.append(z)

                # ---- phase 1: pre_ru^T = sum_s A_s^T-weighted + x0 term ----
                rT = actp.tile([U, N], BF16, tag="rT")
                uT = actp.tile([U, N], F32, tag="uT")
                NC2 = N // 2
                for ic in range(2):
                    p1 = psB.tile([P, NC2], F32, tag="psB")
                    k = 0
                    for s in range(S):
                        for jb in range(JB):
                            nc.tensor.matmul(
                                p1[:],
                                zru[s][:, jb * 2 * U:(jb + 1) * 2 * U],
                                at[s][:, jb * N + ic * NC2: jb * N + (ic + 1) * NC2],
                                start=(k == 0), stop=False)
                            k += 1
                    nc.tensor.matmul(
                        p1[:], w0ru[:], x0Tb[:, ic * NC2:(ic + 1) * NC2],
                        start=False, stop=True)
                    nc.scalar.activation(
                        rT[:, ic * NC2:(ic + 1) * NC2], p1[0:U, :],
                        mybir.ActivationFunctionType.Sigmoid, bias=bias["r"][:])
                    nc.scalar.activation(
                        uT[:, ic * NC2:(ic + 1) * NC2], p1[U:2 * U, :],
                        mybir.ActivationFunctionType.Sigmoid, bias=bias["u"][:])

                # ---- x0c^T = [(r * H)^T | inputs^T] (bf16) ----
                x0cT = actp.tile([F, N], BF16, tag="x0cT")
                nc.vector.tensor_copy(x0cT[U:F, :], x0Tb[U:F, :])
                nc.gpsimd.tensor_mul(x0cT[0:U, :], rT[:], x0T[0:U, :])

                # ---- Z_c_s = x0c @ Wc_{s+1}  (N, 64) bf16 ----
                zc = []
                for s in range(S):
                    z = actp.tile([P, JB * U], BF16, tag=f"zc{s}")
                    for jb in range(JB):
                        pz = psA.tile([P, U], F32, tag="psAx", bufs=2)
                        nc.tensor.matmul(
                            pz[:], x0cT[:, jb * P:(jb + 1) * P], wc_s[s][:],
                            start=True, stop=True)
                        nc.scalar.copy(z[:, jb * U:(jb + 1) * U], pz[:])
                    zc.append(z)

                # ---- phase 2: pre_c^T ----
                cT = actp.tile([U, N], F32, tag="cT")
                for ic in range(2):
                    p2 = psB.tile([U, NC2], F32, tag="psB")
                    k = 0
                    for s in range(S):
                        for jb in range(JB):
                            nc.tensor.matmul(
                                p2[:],
                                zc[s][:, jb * U:(jb + 1) * U],
                                at[s][:, jb * N + ic * NC2: jb * N + (ic + 1) * NC2],
                                start=(k == 0), stop=False)
                            k += 1
                    nc.tensor.matmul(
                        p2[:], wc0[:], x0cT[:, ic * NC2:(ic + 1) * NC2],
                        start=False, stop=True)
                    nc.scalar.activation(
                        cT[:, ic * NC2:(ic + 1) * NC2], p2[:],
                        mybir.ActivationFunctionType.Tanh, bias=bias["c"][:])

                # ---- h^T = c^T + u^T * (H^T - c^T);  back to natural ----
                hT = actp.tile([U, N], F32, tag="hT")
                nc.gpsimd.tensor_sub(hT[:], x0T[0:U, :], cT[:])
                nc.gpsimd.tensor_mul(hT[:], hT[:], uT[:])
                nc.gpsimd.tensor_add(hT[:], hT[:], cT[:])
                hnat = actp.tile([P, JB * U], F32, tag="hnat")
                for jb in range(JB):
                    ph = psA.tile([P, U], F32, tag="psAx", bufs=2)
                    nc.tensor.transpose(
                        ph[:], hT[:, jb * P:(jb + 1) * P], id_f32[0:U, 0:U])
                    nc.scalar.copy(hnat[:, jb * U:(jb + 1) * U], ph[:])
                nc.sync.dma_start(
                    t_out.ap()[b].rearrange("(jb p u) -> p jb u", p=P, u=U),
                    hnat[:].rearrange("p (jb u) -> p jb u", u=U))

    nc.finalize()
    return nc


def _make_in_maps(inputs):
    in_maps = []
    for c in range(NCORES):
        lo, hi = c * BPC, (c + 1) * BPC
        in_maps.append({
            "inputs": np.ascontiguousarray(inputs["inputs"][lo:hi], np.float32),
            "supports": np.ascontiguousarray(inputs["supports"][lo:hi], np.float32),
            "h_prev": np.ascontiguousarray(inputs["h_prev"][lo:hi], np.float32),
            "r_kernel": np.ascontiguousarray(inputs["r_kernel"], np.float32),
            "u_kernel": np.ascontiguousarray(inputs["u_kernel"], np.float32),
            "c_kernel": np.ascontiguousarray(inputs["c_kernel"], np.float32),
            "r_bias": np.ascontiguousarray(inputs["r_bias"], np.float32),
            "u_bias": np.ascontiguousarray(inputs["u_bias"], np.float32),
            "c_bias": np.ascontiguousarray(inputs["c_bias"], np.float32),
        })
    return in_maps


def kernel(**inputs):
    nc = _COMPILED.get("nc")
    if nc is None:
        nc = _COMPILED["nc"] = _build()

    res = run_bass_kernel_spmd(nc, _make_in_maps(inputs), core_ids=list(range(NCORES)))
    out = np.concatenate([res.results[c]["out"] for c in range(NCORES)], axis=0)
    return out.astype(np.float32)


# revision 10
# speedup vs baseline: 1.5337x; 1.1176x over previous
"""GCGRU cell (graph-conv GRU, diffusion-conv gates) on 8 TRN2 NeuronCores.

Math (per batch b, N=1024 nodes, D=2 in-feats, U=64 units, S=2 supports):
  x0   = [H_b | inputs_b]                          (N, 66)  (feature-permuted)
  for gate g in {r, u, c}:
    pre_g = x0g @ Wg_m0 + sum_s A_s @ (x0g @ Wg_{m=s+1}) + bias_g
  (reassociated: (A_s @ x0) @ W == A_s @ (x0 @ W), so the N x N supports
   multiply a tiny (N, 64) matrix instead of the other association order)
  r, u = sigmoid(pre_r), sigmoid(pre_u); c = tanh(pre_c with x0c=[r*H|inputs])
  h = u * H + (1 - u) * c

Implementation notes:
  - Data parallel over batch: 32 batches -> 4 per core, no collectives.
  - supports[b] is cast f32->bf16 *during* the HBM->SBUF DMA (SWDGE cast,
    ~0.86x plain-DMA rate) in natural layout (i on partitions, (j,s) free).
  - The j-contraction needs j on partitions, so A is transposed 128x128
    tile-wise on the TensorEngine (transpose-mode matmul with a bf16
    identity), de-interleaving s via a stride-2 free AP.  Four transposed
    tiles share one [128,512] bf16 PSUM tile (one accumulation group) so
    the VectorEngine drains them in one op.
  - Gate pre-activations accumulate in f32 PSUM with the transposed A_s
    tiles as the moving operand; sigmoid/tanh run on the ScalarEngine
    straight out of PSUM; small drains go to ScalarE, gate elementwise to
    GpSimd, keeping the VectorEngine for the big transpose drains.
"""

import numpy as np

import concourse.bacc as bacc
import concourse.mybir as mybir
import concourse.tile as tile
from concourse.bass_utils import run_bass_kernel_spmd
from concourse.masks import make_identity

B, N, D, U, S = 32, 1024, 2, 64, 2
F = D + U                      # 66
NCORES = 8
BPC = B // NCORES              # 4 batches per core
P = 128                        # partitions
JB = N // P                    # 8 j-blocks per support
F32 = mybir.dt.float32
BF16 = mybir.dt.bfloat16

_COMPILED = {}


def _build():
    nc = bacc.Bacc("TRN2", target_bir_lowering=False, debug=False)

    t_inputs = nc.dram_tensor("inputs", [BPC, N, D], F32, kind="ExternalInput")
    t_supports = nc.dram_tensor("supports", [BPC, N, N, S], F32, kind="ExternalInput")
    t_hprev = nc.dram_tensor("h_prev", [BPC, N * U], F32, kind="ExternalInput")
    t_wk = {g: nc.dram_tensor(f"{g}_kernel", [F * 3, U], F32, kind="ExternalInput")
            for g in "ruc"}
    t_wb = {g: nc.dram_tensor(f"{g}_bias", [U], F32, kind="ExternalInput")
            for g in "ruc"}
    t_out = nc.dram_tensor("out", [BPC, N * U], F32, kind="ExternalOutput")

    QC = 2                 # i-tiles per load chunk
    NCH = N // (QC * P)    # 2 chunks per batch

    with tile.TileContext(nc) as tc:
        with (
            tc.tile_pool(name="const", bufs=1) as constp,
            tc.tile_pool(name="wt", bufs=1) as wtp,
            tc.tile_pool(name="abf", bufs=6) as abfp,
            tc.tile_pool(name="at", bufs=2) as atp,
            tc.tile_pool(name="act", bufs=2) as actp,
            tc.tile_pool(name="psA", bufs=4, space="PSUM") as psA,
            tc.tile_pool(name="psB", bufs=3, space="PSUM") as psB,
        ):
            # ---- constants ----
            id_bf = constp.tile([P, P], BF16, tag="id_bf")
            make_identity(nc, id_bf[:])
            id_f32 = constp.tile([P, P], F32, tag="id_f32")
            make_identity(nc, id_f32[:])

            # ---- gate weights, hop blocks, permuted to [H|inputs], bf16 ----
            # W rows are (f, m) pairs, m fastest: row f*3 + m.
            def load_w_block(g, m, dst_ap):
                st = wtp.tile([F, U], F32, tag=f"wst_{g}{m}", name=f"wst_{g}{m}")
                src = t_wk[g].ap().rearrange("(f three) u -> f three u", three=3)
                nc.sync.dma_start(st[0:U, :], src[D:F, m, :])
                nc.sync.dma_start(st[U:F, :], src[0:D, m, :])
                nc.gpsimd.tensor_copy(dst_ap, st[:])

            w0ru = wtp.tile([F, 2 * U], BF16, tag="w0ru")
            load_w_block("r", 0, w0ru[:, 0:U])
            load_w_block("u", 0, w0ru[:, U:2 * U])
            wru_s = []
            for s in range(S):
                w = wtp.tile([F, 2 * U], BF16, tag=f"wru{s}")
                load_w_block("r", s + 1, w[:, 0:U])
                load_w_block("u", s + 1, w[:, U:2 * U])
                wru_s.append(w)
            wc0 = wtp.tile([F, U], BF16, tag="wc0")
            load_w_block("c", 0, wc0[:])
            wc_s = []
            for s in range(S):
                w = wtp.tile([F, U], BF16, tag=f"wcs{s}")
                load_w_block("c", s + 1, w[:])
                wc_s.append(w)

            bias = {}
            for g in "ruc":
                bt = wtp.tile([U, 1], F32, tag=f"bias_{g}")
                nc.sync.dma_start(bt[:], t_wb[g].ap().rearrange("(u one) -> u one", one=1))
                bias[g] = bt

            sup4 = t_supports.ap().rearrange(
                "b (q p) j two -> b p q (j two)", p=P)

            for b in range(BPC):
                # ---- load (cast f32->bf16 in-DMA) + transpose supports[b] ----
                # AT[s][q, jb*N + i] = A_s[i, jb*128 + q]   (bf16)
                at = [atp.tile([P, JB * N], BF16, tag=f"at{s}", name=f"at{s}")
                      for s in range(S)]
                for ch in range(NCH):
                    ab = abfp.tile([P, QC * N * S], BF16, tag="abf")
                    nc.gpsimd.dma_start(
                        ab[:], sup4[b, :, ch * QC:(ch + 1) * QC, :])
                    ab4 = ab[:].rearrange("p (q j two) -> p q j two", q=QC, two=2)
                    for s in range(S):
                        for jb in range(JB):
                            pt4 = psA.tile([P, QC * P], BF16, tag="psA", bufs=3)
                            for q in range(QC):
                                nc.tensor.matmul(
                                    pt4[:, q * P:(q + 1) * P],
                                    ab4[:, q, jb * P:(jb + 1) * P, s],
                                    id_bf[:],
                                    start=(q == 0), stop=(q == QC - 1),
                                    is_transpose=True)
                            nc.vector.tensor_copy(
                                at[s][:, jb * N + ch * QC * P:
                                      jb * N + (ch + 1) * QC * P],
                                pt4[:])

                # ---- x0 = [H | inputs], transposed (x0T) in f32 + bf16 ----
                x0n = actp.tile([P, JB * F], F32, tag="x0n")
                nc.sync.dma_start(
                    x0n[:].rearrange("p (jb f) -> p jb f", f=F)[:, :, 0:U],
                    t_hprev.ap()[b].rearrange("(jb p u) -> p jb u", p=P, u=U))
                nc.sync.dma_start(
                    x0n[:].rearrange("p (jb f) -> p jb f", f=F)[:, :, U:F],
                    t_inputs.ap()[b].rearrange("(jb p) d -> p jb d", p=P))
                x0T = actp.tile([F, N], F32, tag="x0T")
                x0Tb = actp.tile([F, N], BF16, tag="x0Tb")
                for jb in range(JB):
                    px = psA.tile([F, P], F32, tag="psAx", bufs=2)
                    nc.tensor.transpose(
                        px[:], x0n[:, jb * F:(jb + 1) * F], id_f32[:])
                    nc.scalar.copy(x0T[:, jb * P:(jb + 1) * P], px[:])
                    nc.scalar.copy(x0Tb[:, jb * P:(jb + 1) * P], px[:])

                # ---- Z_ru_s = x0 @ [Wr_{s+1} | Wu_{s+1}]  (N, 128) bf16 ----
                zru = []
                for s in range(S):
                    z = actp.tile([P, JB * 2 * U], BF16, tag=f"zru{s}")
                    for jb in range(JB):
                        pz = psA.tile([P, 2 * U], F32, tag="psAx", bufs=2)
                        nc.tensor.matmul(
                            pz[:], x0Tb[:, jb * P:(jb + 1) * P], wru_s[s][:],
                            start=True, stop=True)
                        nc.scalar.copy(z[:, jb * 2 * U:(jb + 1) * 2 * U], pz[:])
                    zru.append(z)

                # ---- phase 1: pre_ru^T = sum_s A_s^T-weighted + x0 term ----
                rT = actp.tile([U, N], BF16, tag="rT")
                uT = actp.tile([U, N], F32, tag="uT")
                NC2 = N // 2
                for ic in range(2):
                    p1 = psB.tile([P, NC2], F32, tag="psB")
                    k = 0
                    for s in range(S):
                        for jb in range(JB):
                            nc.tensor.matmul(
                                p1[:],
                                zru[s][:, jb * 2 * U:(jb + 1) * 2 * U],
                                at[s][:, jb * N + ic * NC2: jb * N + (ic + 1) * NC2],
                                start=(k == 0), stop=False)
                            k += 1
                    nc.tensor.matmul(
                        p1[:], w0ru[:], x0Tb[:, ic * NC2:(ic + 1) * NC2],
                        start=False, stop=True)
                    nc.scalar.activation(
                        rT[:, ic * NC2:(ic + 1) * NC2], p1[0:U, :],
                        mybir.ActivationFunctionType.Sigmoid, bias=bias["r"][:])
                    nc.scalar.activation(
                        uT[:, ic * NC2:(ic + 1) * NC2], p1[U:2 * U, :],
                        mybir.ActivationFunctionType.Sigmoid, bias=bias["u"][:])

                # ---- x0c^T = [(r * H)^T | inputs^T] (bf16) ----
                x0cT = actp.tile([F, N], BF16, tag="x0cT")
                nc.vector.tensor_copy(x0cT[U:F, :], x0Tb[U:F, :])
                nc.vector.tensor_mul(x0cT[0:U, :], rT[:], x0T[0:U, :])

                # ---- Z_c_s = x0c @ Wc_{s+1}  (N, 64) bf16 ----
                zc = []
                for s in range(S):
                    z = actp.tile([P, JB * U], BF16, tag=f"zc{s}")
                    for jb in range(JB):
                        pz = psA.tile([P, U], F32, tag="psAx", bufs=2)
                        nc.tensor.matmul(
                            pz[:], x0cT[:, jb * P:(jb + 1) * P], wc_s[s][:],
                            start=True, stop=True)
                        nc.scalar.copy(z[:, jb * U:(jb + 1) * U], pz[:])
                    zc.append(z)

                # ---- phase 2: pre_c^T ----
                cT = actp.tile([U, N], F32, tag="cT")
                for ic in range(2):
                    p2 = psB.tile([U, NC2], F32, tag="psB")
                    k = 0
                    for s in range(S):
                        for jb in range(JB):
                            nc.tensor.matmul(
                                p2[:],
                                zc[s][:, jb * U:(jb + 1) * U],
                                at[s][:, jb * N + ic * NC2: jb * N + (ic + 1) * NC2],
                                start=(k == 0), stop=False)
                            k += 1
                    nc.tensor.matmul(
                        p2[:], wc0[:], x0cT[:, ic * NC2:(ic + 1) * NC2],
                        start=False, stop=True)
                    nc.scalar.activation(
                        cT[:, ic * NC2:(ic + 1) * NC2], p2[:],
                        mybir.ActivationFunctionType.Tanh, bias=bias["c"][:])

                # ---- h^T = c^T + u^T * (H^T - c^T);  back to natural ----
                hT = actp.tile([U, N], F32, tag="hT")
                nc.vector.tensor_sub(hT[:], x0T[0:U, :], cT[:])
                nc.vector.tensor_mul(hT[:], hT[:], uT[:])
                nc.vector.tensor_add(hT[:], hT[:], cT[:])
                hnat = actp.tile([P, JB * U], F32, tag="hnat")
                for jb in range(JB):
                    ph = psA.tile([P, U], F32, tag="psAx", bufs=2)
                    nc.tensor.transpose(
                        ph[:], hT[:, jb * P:(jb + 1) * P], id_f32[0:U, 0:U])
                    nc.scalar.copy(hnat[:, jb * U:(jb + 1) * U], ph[:])
                nc.sync.dma_start(
                    t_out.ap()[b].rearrange("(jb p u) -> p jb u", p=P, u=U),
                    hnat[:].rearrange("p (jb u) -> p jb u", u=U))

    nc.finalize()
    return nc


def _make_in_maps(inputs):
    in_maps = []
    for c in range(NCORES):
        lo, hi = c * BPC, (c + 1) * BPC
        in_maps.append({
            "inputs": np.ascontiguousarray(inputs["inputs"][lo:hi], np.float32),
            "supports": np.ascontiguousarray(inputs["supports"][lo:hi], np.float32),
            "h_prev": np.ascontiguousarray(inputs["h_prev"][lo:hi], np.float32),
            "r_kernel": np.ascontiguousarray(inputs["r_kernel"], np.float32),
            "u_kernel": np.ascontiguousarray(inputs["u_kernel"], np.float32),
            "c_kernel": np.ascontiguousarray(inputs["c_kernel"], np.float32),
            "r_bias": np.ascontiguousarray(inputs["r_bias"], np.float32),
            "u_bias": np.ascontiguousarray(inputs["u_bias"], np.float32),
            "c_bias": np.ascontiguousarray(inputs["c_bias"], np.float32),
        })
    return in_maps


def kernel(**inputs):
    nc = _COMPILED.get("nc")
    if nc is None:
        nc = _COMPILED["nc"] = _build()

    res = run_bass_kernel_spmd(nc, _make_in_maps(inputs), core_ids=list(range(NCORES)))
    out = np.concatenate([res.results[c]["out"] for c in range(NCORES)], axis=0)
    return out.astype(np.float32)
